# revision 1
# baseline (speedup 1.0000x reference)
"""CrissCrossAttention kernel for 8x Trainium2 NeuronCores.

Reference computation (B=4, C=512, H=W=64, C8=64):
    q = wq @ x + bq           [B,64,H,W]
    k = wk @ x + bk           [B,64,H,W]
    v = wv @ x + bv           [B,512,H,W]
    energy[(h,w),(k2,v2)] = q[:,h,w] . k[:,k2,v2]        # [B,HW,HW]
    attn = softmax over v2 (inner 64 of the key index)
    out[c,w,v2] = sum_{h,k2} v[c,h,k2] * attn[(h,w),(k2,v2)]
    result = gamma * out + x

Sharding: 8 cores = 4 batches x 2 halves of the query-w axis.  Each core
computes a [512, 32, 64] slice of the output; the host concatenates.
All per-core differences are in input data (SPMD program is identical).
"""

import sys

for p in ("/opt/trn_rl_repo",):
    if p not in sys.path:
        sys.path.insert(0, p)

import numpy as np

import concourse.bass as bass
import concourse.bacc as bacc
import concourse.mybir as mybir
import concourse.tile as tile
from concourse.bass_utils import run_bass_kernel_spmd

F32 = mybir.dt.float32
F32R = mybir.dt.float32r
BF16 = mybir.dt.bfloat16

B, C, H, W = 4, 512, 64, 64
C8 = 64
HW = H * W          # 4096
WHALF = W // 2      # 32
NQ = H * WHALF      # 2048 query positions per core
NCHUNK = 32         # A row chunks of 128 = (h pair) x (64 k2)

_CACHED = {}


def _build_program():
    nc = bacc.Bacc(
        "TRN2",
        target_bir_lowering=False,
        debug=False,
        enable_asserts=False,
        num_devices=8,
    )

    # --- DRAM I/O -------------------------------------------------------
    x_d = nc.dram_tensor("x_full", [C, HW], F32R, kind="ExternalInput").ap()
    xq_d = nc.dram_tensor("x_q", [C, NQ], F32R, kind="ExternalInput").ap()
    xr_d = nc.dram_tensor("x_res", [C, NQ], F32, kind="ExternalInput").ap()
    wqT_d = nc.dram_tensor("wqT", [C, C8], F32R, kind="ExternalInput").ap()
    wkT_d = nc.dram_tensor("wkT", [C, C8], F32R, kind="ExternalInput").ap()
    wvT_d = nc.dram_tensor("wvT", [C, C], F32R, kind="ExternalInput").ap()
    bq_d = nc.dram_tensor("bq", [C8, 1], F32, kind="ExternalInput").ap()
    bk_d = nc.dram_tensor("bk", [C8, 1], F32, kind="ExternalInput").ap()
    bv_d = nc.dram_tensor("bv", [1, C], F32R, kind="ExternalInput").ap()
    g_d = nc.dram_tensor("g128", [128, 1], F32, kind="ExternalInput").ap()
    ones_d = nc.dram_tensor("ones128", [1, 128], F32R, kind="ExternalInput").ap()

    # attn in (h,k2) x (w,v2) layout, bf16:  [chunk s][partition p][w*64+v2]
    a_d = nc.dram_tensor("A_scr", [NCHUNK, 128, NQ], BF16).ap()
    out_d = nc.dram_tensor("out", [C, NQ], F32, kind="ExternalOutput").ap()

    with tile.TileContext(nc) as tc:
        with (
            tc.tile_pool(name="consts", bufs=1) as consts,
            tc.tile_pool(name="qk", bufs=1) as qkp,
            tc.tile_pool(name="vt", bufs=1) as vtp,
            tc.tile_pool(name="dens", bufs=3) as dens,
            tc.tile_pool(name="ps_small", bufs=3, space="PSUM") as ps_small,
            tc.tile_pool(name="ps_out", bufs=1, space="PSUM") as ps_out,
        ):
            # --- constants ---------------------------------------------
            wqT = consts.tile([128, 4 * C8], F32R)
            nc.sync.dma_start(
                wqT[:].rearrange("p (ci o) -> p ci o", ci=4),
                wqT_d.rearrange("(ci p) o -> p ci o", p=128),
            )
            wkT = consts.tile([128, 4 * C8], F32R)
            nc.sync.dma_start(
                wkT[:].rearrange("p (ci o) -> p ci o", ci=4),
                wkT_d.rearrange("(ci p) o -> p ci o", p=128),
            )
            wvT = consts.tile([128, 4 * C], F32R)
            nc.sync.dma_start(
                wvT[:].rearrange("p (ci o) -> p ci o", ci=4),
                wvT_d.rearrange("(ci p) o -> p ci o", p=128),
            )
            bq = consts.tile([C8, 1], F32)
            nc.sync.dma_start(bq[:], bq_d)
            bk = consts.tile([C8, 1], F32)
            nc.sync.dma_start(bk[:], bk_d)
            bv = consts.tile([1, C], F32R)
            nc.sync.dma_start(bv[:], bv_d)
            gsc = consts.tile([128, 1], F32)
            nc.sync.dma_start(gsc[:], g_d)
            ones = consts.tile([1, 128], F32R)
            nc.sync.dma_start(ones[:], ones_d)

            q_sb = qkp.tile([128, NQ], F32R, tag="q")
            k_sb = qkp.tile([128, HW], F32R, tag="k")
            vt_sb = vtp.tile([128, NCHUNK * C], BF16, tag="vt")

            # ==== phase 1: projections (x resident, freed afterwards) ==
            with (
                tc.tile_pool(name="xfull", bufs=1) as xfull,
                tc.tile_pool(name="xqs", bufs=4) as xqs,
            ):
                x_sb = []
                for ci in range(4):
                    xt = xfull.tile([128, HW], F32R, tag=f"x{ci}")
                    nc.sync.dma_start(xt[:], x_d[ci * 128:(ci + 1) * 128, :])
                    x_sb.append(xt)

                # Q projection: Q[64, NQ]
                for n in range(NQ // 512):
                    pq = ps_small.tile([C8, 512], F32, tag="ps")
                    for ci in range(4):
                        xqt = xqs.tile([128, 512], F32R, tag="xqt")
                        nc.sync.dma_start(
                            xqt[:],
                            xq_d[ci * 128:(ci + 1) * 128,
                                 n * 512:(n + 1) * 512],
                        )
                        nc.tensor.matmul(
                            pq[:],
                            wqT[:, ci * C8:(ci + 1) * C8],
                            xqt[:],
                            start=(ci == 0),
                            stop=(ci == 3),
                        )
                    nc.scalar.activation(
                        q_sb[0:C8, n * 512:(n + 1) * 512], pq[:],
                        mybir.ActivationFunctionType.Identity, bias=bq[:],
                    )
                    nc.sync.dma_start(
                        q_sb[C8:128, n * 512:(n + 1) * 512],
                        q_sb[0:C8, n * 512:(n + 1) * 512],
                    )

                # K projection: K[64, HW]
                for n in range(HW // 512):
                    pk = ps_small.tile([C8, 512], F32, tag="ps")
                    for ci in range(4):
                        nc.tensor.matmul(
                            pk[:],
                            wkT[:, ci * C8:(ci + 1) * C8],
                            x_sb[ci][:, n * 512:(n + 1) * 512],
                            start=(ci == 0),
                            stop=(ci == 3),
                        )
                    nc.scalar.activation(
                        k_sb[0:C8, n * 512:(n + 1) * 512], pk[:],
                        mybir.ActivationFunctionType.Identity, bias=bk[:],
                    )
                    nc.sync.dma_start(
                        k_sb[C8:128, n * 512:(n + 1) * 512],
                        k_sb[0:C8, n * 512:(n + 1) * 512],
                    )

                # V^T projection: VT[(h,k2), c] bf16
                # VT[p=hk, c] = sum_c' x[c', hk] * wv[c, c'] + bv[c]
                for s in range(NCHUNK):
                    pv = ps_small.tile([128, C], F32, tag="ps")
                    for ci in range(4):
                        nc.tensor.matmul(
                            pv[:],
                            x_sb[ci][:, s * 128:(s + 1) * 128],
                            wvT[:, ci * C:(ci + 1) * C],
                            start=(ci == 0),
                            stop=False,
                        )
                    nc.tensor.matmul(
                        pv[:], ones[:1, :],
                        bv[:1, :],
                        start=False, stop=True,
                    )
                    nc.scalar.activation(
                        vt_sb[:, s * C:(s + 1) * C], pv[:],
                        mybir.ActivationFunctionType.Copy,
                    )

            # ==== phase 2: energy + softmax + scatter, 16 row tiles ====
            # row tile t: query positions (h = 4t..4t+3) x (32 w)
            with (
                tc.tile_pool(name="exps", bufs=2) as exps,
                tc.tile_pool(name="attn", bufs=2) as attnp,
                tc.tile_pool(name="astream", bufs=6) as astream,
                tc.tile_pool(name="outs", bufs=3) as outsp,
                tc.tile_pool(name="xrs", bufs=4) as xrs,
            ):
                for tp in range(8):
                    # two row tiles run packed in PE row groups 0/1
                    tiles = []
                    for half_idx in range(2):
                        t = 2 * tp + half_idx
                        exp_t = exps.tile([128, HW], BF16, tag="exp")
                        den_t = dens.tile([128, C8], F32, tag="den")
                        tiles.append((t, exp_t, den_t))
                    for n in range(8):
                        pes = []
                        for half_idx in range(2):
                            t, exp_t, den_t = tiles[half_idx]
                            pe = ps_small.tile([128, 512], F32, tag="ps")
                            base = half_idx * C8
                            nc.tensor.matmul(
                                pe[:],
                                q_sb[base:base + C8,
                                     t * 128:(t + 1) * 128],
                                k_sb[base:base + C8,
                                     n * 512:(n + 1) * 512],
                                start=True,
                                stop=True,
                                tile_position=(base, 0),
                            )
                            pes.append(pe)
                        for half_idx in range(2):
                            t, exp_t, den_t = tiles[half_idx]
                            pe = pes[half_idx]
                            nc.scalar.activation(
                                exp_t[:, n * 512:(n + 1) * 512], pe[:],
                                mybir.ActivationFunctionType.Exp,
                            )
                            nc.vector.reduce_sum(
                                den_t[:, n * 8:(n + 1) * 8],
                                exp_t[:, n * 512:(n + 1) * 512].rearrange(
                                    "p (k v) -> p k v", v=64
                                ),
                                axis=mybir.AxisListType.X,
                            )
                    for half_idx in range(2):
                        t, exp_t, den_t = tiles[half_idx]
                        rden_t = dens.tile([128, C8], BF16, tag="rden")
                        with nc.allow_low_precision(reason="softmax recip"):
                            nc.vector.reciprocal(rden_t[:], den_t[:])
                        attn_t = attnp.tile([128, HW], BF16, tag="attn")
                        nc.vector.tensor_mul(
                            attn_t[:].rearrange("p (k v) -> p k v", v=64),
                            exp_t[:].rearrange("p (k v) -> p k v", v=64),
                            rden_t[:].unsqueeze(-1)
                            .broadcast_to([128, C8, 64]),
                        )
                        # scatter to A layout: A[2t+a, b*64+k2, w*64+v2]
                        for a in range(2):
                            for bb in range(2):
                                hlo = 2 * a + bb
                                nc.sync.dma_start(
                                    a_d[2 * t + a, bb * 64:(bb + 1) * 64, :]
                                    .rearrange("k (w v) -> w k v",
                                               w=WHALF, v=64),
                                    attn_t[hlo * 32:(hlo + 1) * 32, :]
                                    .rearrange("w (k v) -> w k v", v=64),
                                )

                # ==== phase 3: out[c, (w,v2)] = VT^T @ A ==============
                # 4 passes: (cpair, nhalf); 4 persistent psum accumulators
                for n4 in range(4):
                    po = []
                    for j in range(4):
                        po_j = ps_out.tile([128, 512], F32, tag=f"po{j}")
                        po.append(po_j)
                    for s in range(NCHUNK):
                        a_sb = astream.tile([128, 512], BF16, tag="astr")
                        nc.sync.dma_start(
                            a_sb[:],
                            a_d[s, :, n4 * 512:(n4 + 1) * 512],
                        )
                        for cg in range(4):
                            nc.tensor.matmul(
                                po[cg][:],
                                vt_sb[:, s * C + cg * 128:
                                      s * C + (cg + 1) * 128],
                                a_sb[:],
                                start=(s == 0),
                                stop=(s == NCHUNK - 1),
                            )
                    for cg in range(4):
                        col = n4 * 512
                        xrt = xrs.tile([128, 512], F32, tag="xrt")
                        nc.sync.dma_start(
                            xrt[:],
                            xr_d[cg * 128:(cg + 1) * 128, col:col + 512],
                        )
                        o_sb = outsp.tile([128, 512], F32, tag="osb")
                        nc.vector.scalar_tensor_tensor(
                            o_sb[:],
                            po[cg][:],
                            gsc[:],
                            xrt[:],
                            op0=mybir.AluOpType.mult,
                            op1=mybir.AluOpType.add,
                        )
                        nc.sync.dma_start(
                            out_d[cg * 128:(cg + 1) * 128, col:col + 512],
                            o_sb[:],
                        )

    nc.compile()
    return nc


def _host_prep(x, wq, bq, wk, bk, wv, bv, gamma):
    wqT = np.ascontiguousarray(wq.T)
    wkT = np.ascontiguousarray(wk.T)
    wvT = np.ascontiguousarray(wv.T)
    bq2 = np.ascontiguousarray(bq.reshape(C8, 1))
    bk2 = np.ascontiguousarray(bk.reshape(C8, 1))
    bv2 = np.ascontiguousarray(bv.reshape(1, C))
    g128 = np.ascontiguousarray(
        np.broadcast_to(gamma.reshape(1, 1), (128, 1)).astype(np.float32)
    )
    in_maps = []
    for core in range(8):
        b = core // 2
        half = core % 2
        xb = x[b]                                    # [C, H, W]
        in_maps.append({
            "x_full": np.ascontiguousarray(xb.reshape(C, HW)),
            "x_q": np.ascontiguousarray(
                xb[:, :, half * WHALF:(half + 1) * WHALF].reshape(C, NQ)
            ),
            "x_res": np.ascontiguousarray(
                xb[:, half * WHALF:(half + 1) * WHALF, :].reshape(C, NQ)
            ),
            "wqT": wqT, "wkT": wkT, "wvT": wvT,
            "bq": bq2, "bk": bk2, "bv": bv2, "g128": g128,
            "ones128": np.ones((1, 128), dtype=np.float32),
        })
    return in_maps


def kernel(x, wq, bq, wk, bk, wv, bv, gamma):
    x = np.ascontiguousarray(np.asarray(x, dtype=np.float32))
    wq = np.asarray(wq, dtype=np.float32)
    wk = np.asarray(wk, dtype=np.float32)
    wv = np.asarray(wv, dtype=np.float32)
    bq = np.asarray(bq, dtype=np.float32)
    bk = np.asarray(bk, dtype=np.float32)
    bv = np.asarray(bv, dtype=np.float32)
    gamma = np.asarray(gamma, dtype=np.float32)

    if "nc" not in _CACHED:
        _CACHED["nc"] = _build_program()
    nc = _CACHED["nc"]

    in_maps = _host_prep(x, wq, bq, wk, bk, wv, bv, gamma)
    _CACHED["in_maps"] = in_maps

    results = run_bass_kernel_spmd(nc, in_maps, list(range(8))).results

    out = np.empty((B, C, H, W), dtype=np.float32)
    for core in range(8):
        b = core // 2
        half = core % 2
        o = results[core]["out"].reshape(C, WHALF, W)
        out[b, :, half * WHALF:(half + 1) * WHALF, :] = o
    return out



# revision 2
# speedup vs baseline: 42.0597x; 42.0597x over previous
"""CrissCrossAttention kernel for 8x Trainium2 NeuronCores.

Reference computation (B=4, C=512, H=W=64, C8=64):
    q = wq @ x + bq           [B,64,H,W]
    k = wk @ x (+bk cancels)  [B,64,H,W]
    v = wv @ x + bv           [B,512,H,W]
    energy[(h,w),(k2,v2)] = q[:,h,w] . k[:,k2,v2]        # [B,HW,HW]
    attn = softmax over v2 (inner 64 of the key index)
    out[c,w,v2] = sum_{h,k2} v[c,h,k2] * attn[(h,w),(k2,v2)]
    result = gamma * out + x

Notes:
  - bk is dropped: it adds q.bk, constant across the softmax axis (v2), so
    it cancels in the softmax.
  - Everything upstream of the f32 PSUM accumulations runs in bf16
    (validated: rel err ~4e-3 vs f32 reference, budget 2e-2).
  - The attention matrix A[(h,k2),(w,v2)] stays resident in SBUF (16.8MB);
    the (h,w,k2,v2)->((h,k2),(w,v2)) shuffle is one SBUF->SBUF DMA per
    128-query row tile.
  - Phase-3 pass A (output cols 0-511) accumulates A chunks as they are
    produced, interleaved with the softmax pipeline; passes B-D run after.

Sharding: 8 cores = 4 batches x 2 halves of the query-w axis.  Each core
computes a [512, 32, 64] slice of the output; the host concatenates.
"""

import sys

for p in ("/opt/trn_rl_repo",):
    if p not in sys.path:
        sys.path.insert(0, p)

import numpy as np

import concourse.bass as bass
import concourse.bacc as bacc
import concourse.mybir as mybir
import concourse.tile as tile
from concourse.bass_utils import run_bass_kernel_spmd

F32 = mybir.dt.float32
BF16 = mybir.dt.bfloat16

B, C, H, W = 4, 512, 64, 64
C8 = 64
HW = H * W          # 4096
WHALF = W // 2      # 32
NQ = H * WHALF      # 2048 query positions per core

_CACHED = {}


def _build_program():
    nc = bacc.Bacc(
        "TRN2",
        target_bir_lowering=False,
        debug=False,
        enable_asserts=False,
        num_devices=8,
    )

    # --- DRAM I/O -------------------------------------------------------
    x_d = nc.dram_tensor("x_full", [C, HW], BF16, kind="ExternalInput").ap()
    xq_d = nc.dram_tensor("x_q", [C, NQ], BF16, kind="ExternalInput").ap()
    xr_d = nc.dram_tensor("x_res", [C, NQ], F32, kind="ExternalInput").ap()
    wqT_d = nc.dram_tensor("wqT", [C, C8], BF16, kind="ExternalInput").ap()
    wkT_d = nc.dram_tensor("wkT", [C, C8], BF16, kind="ExternalInput").ap()
    wvT_d = nc.dram_tensor("wvT", [C, C], BF16, kind="ExternalInput").ap()
    bq_d = nc.dram_tensor("bq", [C8, 1], F32, kind="ExternalInput").ap()
    bv_d = nc.dram_tensor("bv", [1, C], BF16, kind="ExternalInput").ap()
    g_d = nc.dram_tensor("g128", [128, 1], F32, kind="ExternalInput").ap()
    out_d = nc.dram_tensor("out", [C, NQ], F32, kind="ExternalOutput").ap()

    with tile.TileContext(nc) as tc:
        with (
            tc.tile_pool(name="consts", bufs=1) as consts,
            tc.tile_pool(name="qk", bufs=1) as qkp,
            tc.tile_pool(name="vt", bufs=1) as vtp,
            tc.tile_pool(name="abuf", bufs=1) as abuf,
            tc.tile_pool(name="exps", bufs=2) as exps,
            tc.tile_pool(name="dens", bufs=3) as dens,
            tc.tile_pool(name="outs", bufs=2) as outsp,
            tc.tile_pool(name="xrs", bufs=2) as xrs,
            tc.tile_pool(name="ps_small", bufs=3, space="PSUM") as ps_small,
            tc.tile_pool(name="ps_out", bufs=1, space="PSUM") as ps_out,
        ):
            # --- constants ---------------------------------------------
            wqT = consts.tile([128, 4 * C8], BF16)
            nc.sync.dma_start(
                wqT[:].rearrange("p (ci o) -> p ci o", ci=4),
                wqT_d.rearrange("(ci p) o -> p ci o", p=128),
            )
            wkT = consts.tile([128, 4 * C8], BF16)
            nc.sync.dma_start(
                wkT[:].rearrange("p (ci o) -> p ci o", ci=4),
                wkT_d.rearrange("(ci p) o -> p ci o", p=128),
            )
            wvT = consts.tile([128, 4 * C], BF16)
            nc.sync.dma_start(
                wvT[:].rearrange("p (ci o) -> p ci o", ci=4),
                wvT_d.rearrange("(ci p) o -> p ci o", p=128),
            )
            bq = consts.tile([C8, 1], F32)
            nc.sync.dma_start(bq[:], bq_d)
            bv = consts.tile([1, C], BF16)
            nc.sync.dma_start(bv[:], bv_d)
            gsc = consts.tile([128, 1], F32)
            nc.sync.dma_start(gsc[:], g_d)

            q_sb = qkp.tile([128, NQ], BF16, tag="q")
            k_sb = qkp.tile([128, HW], BF16, tag="k")
            vt_sb = vtp.tile([128, 32 * C], BF16, tag="vt")
            a_sb = [abuf.tile([128, HW], BF16, tag=f"a{t}") for t in range(16)]

            # ==== phase 1: projections (x resident, freed afterwards) ==
            with tc.tile_pool(name="xfull", bufs=1) as xfull:
                x_sb = []
                for ci in range(4):
                    xt = xfull.tile([128, HW], BF16, tag=f"x{ci}")
                    nc.sync.dma_start(xt[:], x_d[ci * 128:(ci + 1) * 128, :])
                    x_sb.append(xt)
                xq_sb = []
                for ci in range(4):
                    xqt = xfull.tile([128, NQ], BF16, tag=f"xq{ci}")
                    nc.sync.dma_start(xqt[:], xq_d[ci * 128:(ci + 1) * 128, :])
                    xq_sb.append(xqt)

                # V^T projection: VT[p=(hh,k2), c] bf16, chunk s = h-pair
                for s in range(32):
                    pv = ps_small.tile([128, C], F32, tag="ps")
                    for ci in range(4):
                        nc.tensor.matmul(
                            pv[:],
                            x_sb[ci][:, s * 128:(s + 1) * 128],
                            wvT[:, ci * C:(ci + 1) * C],
                            start=(ci == 0),
                            stop=(ci == 3),
                        )
                    nc.scalar.activation(
                        vt_sb[:, s * C:(s + 1) * C], pv[:],
                        mybir.ActivationFunctionType.Copy,
                    )
                # add bv (broadcast over partitions and chunks) in one DVE op
                with nc.allow_low_precision(reason="v bias add bf16"):
                    nc.vector.tensor_add(
                        vt_sb[:].rearrange("p (s c) -> p s c", s=32),
                        vt_sb[:].rearrange("p (s c) -> p s c", s=32),
                        bv[:1, :].broadcast_to([128, C])
                        .unsqueeze(1).broadcast_to([128, 32, C]),
                    )

                # Q projection: Q[64, NQ] += bq
                for n in range(NQ // 512):
                    pq = ps_small.tile([C8, 512], F32, tag="ps")
                    for ci in range(4):
                        nc.tensor.matmul(
                            pq[:],
                            wqT[:, ci * C8:(ci + 1) * C8],
                            xq_sb[ci][:, n * 512:(n + 1) * 512],
                            start=(ci == 0),
                            stop=(ci == 3),
                        )
                    nc.scalar.activation(
                        q_sb[0:C8, n * 512:(n + 1) * 512], pq[:],
                        mybir.ActivationFunctionType.Identity, bias=bq[:],
                    )
                nc.sync.dma_start(q_sb[C8:128, :], q_sb[0:C8, :])

                # K projection: K[64, HW] (no bias: cancels in softmax)
                for n in range(HW // 512):
                    pk = ps_small.tile([C8, 512], F32, tag="ps")
                    for ci in range(4):
                        nc.tensor.matmul(
                            pk[:],
                            wkT[:, ci * C8:(ci + 1) * C8],
                            x_sb[ci][:, n * 512:(n + 1) * 512],
                            start=(ci == 0),
                            stop=(ci == 3),
                        )
                    nc.scalar.activation(
                        k_sb[0:C8, n * 512:(n + 1) * 512], pk[:],
                        mybir.ActivationFunctionType.Copy,
                    )
                nc.sync.dma_start(k_sb[C8:128, :], k_sb[0:C8, :])

            # ==== phase 2 + phase-3 pass A =============================
            # pass-A psum accumulators: out cols 0..511, 4 channel groups
            po_a = [ps_out.tile([128, 512], F32, tag=f"po{cg}") for cg in range(4)]

            for tp in range(8):
                tiles = []
                for half_idx in range(2):
                    t = 2 * tp + half_idx
                    exp_t = exps.tile([128, HW], BF16, tag="exp")
                    den_t = dens.tile([128, C8], BF16, tag="den")
                    tiles.append((t, exp_t, den_t))
                for n in range(8):
                    pes = []
                    for half_idx in range(2):
                        t, exp_t, den_t = tiles[half_idx]
                        pe = ps_small.tile([128, 512], F32, tag="ps")
                        base = half_idx * C8
                        nc.tensor.matmul(
                            pe[:],
                            q_sb[base:base + C8, t * 128:(t + 1) * 128],
                            k_sb[base:base + C8, n * 512:(n + 1) * 512],
                            start=True,
                            stop=True,
                            tile_position=(base, 0),
                        )
                        pes.append(pe)
                    for half_idx in range(2):
                        t, exp_t, den_t = tiles[half_idx]
                        pe = pes[half_idx]
                        nc.scalar.activation(
                            exp_t[:, n * 512:(n + 1) * 512], pe[:],
                            mybir.ActivationFunctionType.Exp,
                        )
                        with nc.allow_low_precision(reason="softmax den bf16"):
                            nc.vector.reduce_sum(
                                den_t[:, n * 8:(n + 1) * 8],
                                exp_t[:, n * 512:(n + 1) * 512].rearrange(
                                    "p (k v) -> p k v", v=64
                                ),
                                axis=mybir.AxisListType.X,
                            )
                for half_idx in range(2):
                    t, exp_t, den_t = tiles[half_idx]
                    rden_t = dens.tile([128, C8], BF16, tag="rden")
                    with nc.allow_low_precision(reason="softmax recip"):
                        nc.vector.reciprocal(rden_t[:], den_t[:])
                    # normalize in place on the Pool engine
                    nc.gpsimd.tensor_mul(
                        exp_t[:].rearrange("p (k v) -> p k v", v=64),
                        exp_t[:].rearrange("p (k v) -> p k v", v=64),
                        rden_t[:].unsqueeze(-1).broadcast_to([128, C8, 64]),
                    )
                    # scatter to A layout in one SBUF->SBUF DMA:
                    # a_sb[t][p=(b k2), f=(a w v2)] <- exp[p=(a b w), f=(k2 v2)]
                    nc.sync.dma_start(
                        a_sb[t][:].rearrange(
                            "(b k) (a w v) -> b k a w v", b=2, a=2, w=WHALF, v=64
                        ),
                        exp_t[:].rearrange(
                            "(a b w) (k v) -> b k a w v", a=2, b=2, w=WHALF, v=64
                        ),
                    )
                    # phase-3 pass A: accumulate chunks 2t, 2t+1
                    for u in range(2):
                        s = 2 * t + u
                        for cg in range(4):
                            nc.tensor.matmul(
                                po_a[cg][:],
                                vt_sb[:, s * C + cg * 128:s * C + (cg + 1) * 128],
                                a_sb[t][:, u * 2048:u * 2048 + 512],
                                start=(s == 0),
                                stop=(s == 31),
                            )

            # ==== pass-A writeback + phase-3 passes B..D ===============
            def writeback(po, j):
                for cg in range(4):
                    col = j * 512
                    xrt = xrs.tile([128, 512], F32, tag="xrt")
                    nc.scalar.dma_start(
                        xrt[:],
                        xr_d[cg * 128:(cg + 1) * 128, col:col + 512],
                    )
                    o_sb = outsp.tile([128, 512], F32, tag="osb")
                    nc.vector.scalar_tensor_tensor(
                        o_sb[:],
                        po[cg][:],
                        gsc[:],
                        xrt[:],
                        op0=mybir.AluOpType.mult,
                        op1=mybir.AluOpType.add,
                    )
                    nc.scalar.dma_start(
                        out_d[cg * 128:(cg + 1) * 128, col:col + 512],
                        o_sb[:],
                    )

            writeback(po_a, 0)

            for j in range(1, 4):
                po = [ps_out.tile([128, 512], F32, tag=f"po{cg}") for cg in range(4)]
                for s in range(32):
                    t, u = s // 2, s % 2
                    for cg in range(4):
                        nc.tensor.matmul(
                            po[cg][:],
                            vt_sb[:, s * C + cg * 128:s * C + (cg + 1) * 128],
                            a_sb[t][:, u * 2048 + j * 512:u * 2048 + (j + 1) * 512],
                            start=(s == 0),
                            stop=(s == 31),
                        )
                writeback(po, j)

    nc.compile()
    return nc


def _host_prep(x, wq, bq, wk, bk, wv, bv, gamma):
    bf = mybir.dt.np(BF16)
    wqT = np.ascontiguousarray(wq.T).astype(bf)
    wkT = np.ascontiguousarray(wk.T).astype(bf)
    wvT = np.ascontiguousarray(wv.T).astype(bf)
    bq2 = np.ascontiguousarray(bq.reshape(C8, 1))
    bv2 = np.ascontiguousarray(bv.reshape(1, C)).astype(bf)
    g128 = np.ascontiguousarray(
        np.broadcast_to(gamma.reshape(1, 1), (128, 1)).astype(np.float32)
    )
    in_maps = []
    xb_bf = [x[b].reshape(C, HW).astype(bf) for b in range(B)]
    for core in range(8):
        b = core // 2
        half = core % 2
        xb = x[b]                                    # [C, H, W]
        in_maps.append({
            "x_full": xb_bf[b],
            "x_q": np.ascontiguousarray(
                xb[:, :, half * WHALF:(half + 1) * WHALF].reshape(C, NQ)
            ).astype(bf),
            "x_res": np.ascontiguousarray(
                xb[:, half * WHALF:(half + 1) * WHALF, :].reshape(C, NQ)
            ),
            "wqT": wqT, "wkT": wkT, "wvT": wvT,
            "bq": bq2, "bv": bv2, "g128": g128,
        })
    return in_maps


def kernel(x, wq, bq, wk, bk, wv, bv, gamma):
    x = np.ascontiguousarray(np.asarray(x, dtype=np.float32))
    wq = np.asarray(wq, dtype=np.float32)
    wk = np.asarray(wk, dtype=np.float32)
    wv = np.asarray(wv, dtype=np.float32)
    bq = np.asarray(bq, dtype=np.float32)
    bv = np.asarray(bv, dtype=np.float32)
    gamma = np.asarray(gamma, dtype=np.float32)

    if "nc" not in _CACHED:
        _CACHED["nc"] = _build_program()
    nc = _CACHED["nc"]

    in_maps = _host_prep(x, wq, bq, wk, bk, wv, bv, gamma)
    _CACHED["in_maps"] = in_maps

    results = run_bass_kernel_spmd(nc, in_maps, list(range(8))).results

    out = np.empty((B, C, H, W), dtype=np.float32)
    for core in range(8):
        b = core // 2
        half = core % 2
        o = results[core]["out"].reshape(C, WHALF, W)
        out[b, :, half * WHALF:(half + 1) * WHALF, :] = o
    return out
